# revision 7
# baseline (speedup 1.0000x reference)
"""ClusterMultiHeadedAttention Trainium2 kernel.

Strategy: cluster-grouped block-diagonal attention.
- Host groups tokens by cluster label (queries by query_labels, keys/values
  by value_labels) into a padded static layout shared by all 8 cores
  (per-cluster capacity = max count over batches, SPMD-safe).
- 8 cores = 4 batches x 2 head-groups (2 heads each). Per core: Q/K/V
  projections, per-cluster attention (scores only within a cluster block),
  masked softmax via a vmask column fused into the V matrix (rowsum lands in
  the PV matmul output), normalization via a PE ones-broadcast matmul, and a
  local partial merge GEMM over the core's 2 heads.
- Host sums the two partial merges per batch, adds bm, un-permutes columns.
"""

import sys

sys.path.insert(0, "/opt/trn_rl_repo")

import numpy as np

D_MODEL = 256
N_HEADS = 4
HEAD_DIM = 64
N_CLUSTERS = 16
B = 4
N_TOK = 2048
P = 128
FP32 = None  # set after mybir import

_PROG_CACHE = {}


def _head_rows(h):
    # reference reshape [256] -> [64, 4]: d = hd*4 + h
    return np.arange(HEAD_DIM) * N_HEADS + h


def _positions(lab, off):
    """Padded position for each token given cluster offsets `off` [17]."""
    n = lab.shape[0]
    order = np.argsort(lab, kind="stable")
    sl = lab[order]
    first = np.searchsorted(sl, sl, side="left")
    within = np.arange(n) - first
    pos = np.empty(n, dtype=np.int64)
    pos[order] = off[sl] + within
    return pos


def _build_program(NQ, NK, Pq, Pk):
    import concourse.bacc as bacc
    import concourse.mybir as mybir
    import concourse.tile as tile

    f32 = mybir.dt.float32
    AF = mybir.ActivationFunctionType

    qoff = np.concatenate([[0], np.cumsum(Pq)]).astype(int)
    koff = np.concatenate([[0], np.cumsum(Pk)]).astype(int)

    nc = bacc.Bacc("TRN2", target_bir_lowering=False, debug=False, num_devices=8)

    xq = nc.declare_dram_parameter("xq", [D_MODEL, NQ], f32, isOutput=False)
    xk = nc.declare_dram_parameter("xk", [D_MODEL, NK], f32, isOutput=False)
    xv = nc.declare_dram_parameter("xv", [3 * P, NK], f32, isOutput=False)
    wq = nc.declare_dram_parameter("wq", [D_MODEL, P], f32, isOutput=False)
    wk = nc.declare_dram_parameter("wk", [D_MODEL, P], f32, isOutput=False)
    wv = nc.declare_dram_parameter("wv", [3 * P, 130], f32, isOutput=False)
    wm0 = nc.declare_dram_parameter("wm0", [HEAD_DIM, D_MODEL], f32, isOutput=False)
    wm1 = nc.declare_dram_parameter("wm1", [HEAD_DIM, D_MODEL], f32, isOutput=False)
    bqp = nc.declare_dram_parameter("bq", [P, 1], f32, isOutput=False)
    bkp = nc.declare_dram_parameter("bk", [P, 1], f32, isOutput=False)
    out = nc.declare_dram_parameter("out", [D_MODEL, NQ], f32, isOutput=True)

    qwins = [(s, min(512, NQ - s)) for s in range(0, NQ, 512)]
    kwins_p = [(s, min(512, NK - s)) for s in range(0, NK, 512)]

    def kwins(c):
        w = [(0, min(P, Pk[c]))]
        if Pk[c] > P:
            w.append((P, Pk[c] - P))
        return w

    with tile.TileContext(nc) as tc:
        with (
            tc.tile_pool(name="const", bufs=1) as cpool,
            tc.tile_pool(name="inp", bufs=1) as ipool,
            tc.tile_pool(name="big", bufs=1) as bpool,
            tc.tile_pool(name="ex", bufs=4) as expool,
            tc.tile_pool(name="small", bufs=4) as spool,
            tc.tile_pool(name="bcp", bufs=3) as bcpool,
            tc.tile_pool(name="osb", bufs=3) as opool,
        ):
            # ---- constants / weights ----
            wq_sb = cpool.tile([P, 2 * P], f32, tag="wq")
            wk_sb = cpool.tile([P, 2 * P], f32, tag="wk")
            wv_sb = cpool.tile([P, 3 * 130], f32, tag="wv")
            wm0_sb = cpool.tile([HEAD_DIM, D_MODEL], f32, tag="wm0")
            wm1_sb = cpool.tile([HEAD_DIM, D_MODEL], f32, tag="wm1")
            bq_sb = cpool.tile([P, 1], f32, tag="bq")
            bk_sb = cpool.tile([P, 1], f32, tag="bk")
            ones_sb = cpool.tile([P, HEAD_DIM], f32, tag="ones")
            for r in range(2):
                nc.sync.dma_start(out=wq_sb[:, r * P:(r + 1) * P], in_=wq[r * P:(r + 1) * P, :])
                nc.sync.dma_start(out=wk_sb[:, r * P:(r + 1) * P], in_=wk[r * P:(r + 1) * P, :])
            for r in range(3):
                nc.sync.dma_start(out=wv_sb[:, r * 130:(r + 1) * 130], in_=wv[r * P:(r + 1) * P, :])
            nc.sync.dma_start(out=wm0_sb[:], in_=wm0[:])
            nc.sync.dma_start(out=wm1_sb[:], in_=wm1[:])
            nc.sync.dma_start(out=bq_sb[:], in_=bqp[:])
            nc.sync.dma_start(out=bk_sb[:], in_=bkp[:])
            nc.vector.memset(ones_sb[:], 1.0)

            # ---- input tiles (DMA per window for overlap) ----
            xq_t = {}
            xk_t = {}
            for r in range(2):
                for wi, (s, w) in enumerate(qwins):
                    t = ipool.tile([P, w], f32, tag=f"xq{r}_{wi}", name=f"xq{r}_{wi}")
                    nc.sync.dma_start(out=t[:], in_=xq[r * P:(r + 1) * P, s:s + w])
                    xq_t[r, wi] = t
                for wi, (s, w) in enumerate(kwins_p):
                    t = ipool.tile([P, w], f32, tag=f"xk{r}_{wi}", name=f"xk{r}_{wi}")
                    nc.sync.dma_start(out=t[:], in_=xk[r * P:(r + 1) * P, s:s + w])
                    xk_t[r, wi] = t
            xv_t = {}
            for r in range(3):
                for c in range(N_CLUSTERS):
                    t = ipool.tile([P, Pk[c]], f32, tag=f"xv{r}_{c}", name=f"xv{r}_{c}")
                    nc.sync.dma_start(
                        out=t[:], in_=xv[r * P:(r + 1) * P, koff[c]:koff[c] + Pk[c]]
                    )
                    xv_t[r, c] = t

            # ---- resident intermediates ----
            q2h = bpool.tile([P, NQ], f32, tag="q2h")
            k2h = bpool.tile([P, NK], f32, tag="k2h")
            nv0 = bpool.tile([HEAD_DIM, NQ], f32, tag="nv0")
            nv1 = bpool.tile([HEAD_DIM, NQ], f32, tag="nv1")
            nv = [nv0, nv1]
            vt = {}
            for c in range(N_CLUSTERS):
                for j, (wo, wc) in enumerate(kwins(c)):
                    vt[c, j] = bpool.tile([wc, 130], f32, tag=f"vt{c}_{j}", name=f"vt{c}_{j}")

            # ---- Q/K projections ----
            with tc.tile_pool(name="pps", bufs=4, space="PSUM") as pps:
                for wi, (s, w) in enumerate(qwins):
                    ps = pps.tile([P, w], f32, tag="qk")
                    for r in range(2):
                        nc.tensor.matmul(
                            ps[:], wq_sb[:, r * P:(r + 1) * P], xq_t[r, wi][:],
                            start=(r == 0), stop=(r == 1),
                        )
                    nc.scalar.activation(q2h[:, s:s + w], ps[:], AF.Identity, bias=bq_sb[:])
                for wi, (s, w) in enumerate(kwins_p):
                    ps = pps.tile([P, w], f32, tag="qk")
                    for r in range(2):
                        nc.tensor.matmul(
                            ps[:], wk_sb[:, r * P:(r + 1) * P], xk_t[r, wi][:],
                            start=(r == 0), stop=(r == 1),
                        )
                    nc.scalar.activation(k2h[:, s:s + w], ps[:], AF.Identity, bias=bk_sb[:])

                # ---- V projection (token-partition layout, mask col fused) ----
                for c in range(N_CLUSTERS):
                    for j, (wo, wc) in enumerate(kwins(c)):
                        ps = pps.tile([P, 130], f32, tag="v")
                        for r in range(3):
                            nc.tensor.matmul(
                                ps[:wc, :], xv_t[r, c][:, wo:wo + wc],
                                wv_sb[:, r * 130:(r + 1) * 130],
                                start=(r == 0), stop=(r == 2),
                            )
                        nc.vector.tensor_copy(vt[c, j][:], ps[:wc, :])

            # ---- attention per (head, cluster) ----
            with (
                tc.tile_pool(name="stps", bufs=3, space="PSUM") as stps,
                tc.tile_pool(name="pvps", bufs=2, space="PSUM") as pvps,
                tc.tile_pool(name="bcps", bufs=2, space="PSUM") as bcps,
            ):
                for c in range(N_CLUSTERS):
                    nq = int(Pq[c])
                    qo = int(qoff[c])
                    for h in range(2):
                        hp = h * HEAD_DIM
                        ex = {}
                        for j, (wo, wc) in enumerate(kwins(c)):
                            st = stps.tile([P, nq], f32, tag="st")
                            nc.tensor.matmul(
                                st[:wc, :],
                                k2h[hp:hp + HEAD_DIM, koff[c] + wo:koff[c] + wo + wc],
                                q2h[hp:hp + HEAD_DIM, qo:qo + nq],
                                start=True, stop=True,
                            )
                            e = expool.tile([P, nq], f32, tag="ex")
                            nc.scalar.activation(e[:wc, :], st[:wc, :], AF.Exp, scale=0.125)
                            ex[j] = (e, wc)
                        pv = pvps.tile([HEAD_DIM + 1, nq], f32, tag="pv")
                        nw = len(ex)
                        for j, (e, wc) in ex.items():
                            nc.tensor.matmul(
                                pv[:], vt[c, j][:, h * 65:h * 65 + 65], e[:wc, :],
                                start=(j == 0), stop=(j == nw - 1),
                            )
                        # rowsum is pv row 64; recip chain stays on partition 64
                        rs = spool.tile([HEAD_DIM + 1, nq], f32, tag="rs")
                        nc.vector.tensor_scalar_add(
                            rs[HEAD_DIM:HEAD_DIM + 1, :], pv[HEAD_DIM:HEAD_DIM + 1, :], 1e-30
                        )
                        rc = spool.tile([HEAD_DIM + 1, nq], f32, tag="rc")
                        nc.vector.reciprocal(
                            rc[HEAD_DIM:HEAD_DIM + 1, :], rs[HEAD_DIM:HEAD_DIM + 1, :]
                        )
                        bc = bcps.tile([HEAD_DIM, nq], f32, tag="bc")
                        nc.tensor.matmul(
                            bc[:], ones_sb[HEAD_DIM:HEAD_DIM + 1, :],
                            rc[HEAD_DIM:HEAD_DIM + 1, :],
                            start=True, stop=True,
                        )
                        bs = bcpool.tile([HEAD_DIM, nq], f32, tag="bs")
                        nc.scalar.activation(bs[:], bc[:], AF.Copy)
                        nc.vector.tensor_mul(
                            nv[h][:, qo:qo + nq], pv[:HEAD_DIM, :], bs[:]
                        )

            # ---- merge (partial over this core's 2 heads) ----
            with tc.tile_pool(name="mps", bufs=3, space="PSUM") as mps:
                for oc in range(2):
                    for wi, (s, w) in enumerate(qwins):
                        ps = mps.tile([P, w], f32, tag="m")
                        nc.tensor.matmul(
                            ps[:], wm0_sb[:, oc * P:(oc + 1) * P], nv0[:, s:s + w],
                            start=True, stop=False,
                        )
                        nc.tensor.matmul(
                            ps[:], wm1_sb[:, oc * P:(oc + 1) * P], nv1[:, s:s + w],
                            start=False, stop=True,
                        )
                        ot = opool.tile([P, w], f32, tag="o")
                        nc.scalar.activation(ot[:], ps[:], AF.Copy)
                        nc.sync.dma_start(out=out[oc * P:(oc + 1) * P, s:s + w], in_=ot[:])

    nc.compile()
    return nc


def _prepare(query, key, value, query_labels, value_labels,
             Wq, bq, Wk, bk, Wv, bv, Wm, bm):
    qlab = np.asarray(query_labels).astype(np.int64)
    vlab = np.asarray(value_labels).astype(np.int64)
    query = np.asarray(query, dtype=np.float32)
    key = np.asarray(key, dtype=np.float32)
    value = np.asarray(value, dtype=np.float32)
    Wq, bq = np.asarray(Wq, np.float32), np.asarray(bq, np.float32)
    Wk, bk = np.asarray(Wk, np.float32), np.asarray(bk, np.float32)
    Wv, bv = np.asarray(Wv, np.float32), np.asarray(bv, np.float32)
    Wm, bm = np.asarray(Wm, np.float32), np.asarray(bm, np.float32)

    qc = np.stack([np.bincount(qlab[b], minlength=N_CLUSTERS) for b in range(B)])
    vc = np.stack([np.bincount(vlab[b], minlength=N_CLUSTERS) for b in range(B)])
    Pq = ((np.maximum(qc.max(0), 4) + 3) // 4 * 4).astype(int)
    Pk = ((np.maximum(vc.max(0), 4) + 3) // 4 * 4).astype(int)
    assert Pq.max() <= 512 and Pk.max() <= 256, (Pq.max(), Pk.max())
    qoff = np.concatenate([[0], np.cumsum(Pq)]).astype(int)
    koff = np.concatenate([[0], np.cumsum(Pk)]).astype(int)
    NQ, NK = int(qoff[-1]), int(koff[-1])

    qpos = [ _positions(qlab[b], qoff) for b in range(B) ]
    vpos = [ _positions(vlab[b], koff) for b in range(B) ]

    in_maps = []
    for core in range(8):
        b, g = core // 2, core % 2
        h0, h1 = 2 * g, 2 * g + 1
        r0, r1 = _head_rows(h0), _head_rows(h1)
        rows2h = np.concatenate([r0, r1])

        xq = np.zeros((D_MODEL, NQ), np.float32)
        xq[:, qpos[b]] = query[b]
        xk = np.zeros((D_MODEL, NK), np.float32)
        xk[:, vpos[b]] = key[b]
        xv = np.zeros((3 * P, NK), np.float32)
        xv[:D_MODEL][:, vpos[b]] = value[b]
        xv[D_MODEL, vpos[b]] = 1.0  # vmask

        wv = np.zeros((3 * P, 130), np.float32)
        wv[:D_MODEL, 0:64] = Wv[r0].T
        wv[:D_MODEL, 65:129] = Wv[r1].T
        wv[D_MODEL, 0:64] = bv[r0]
        wv[D_MODEL, 65:129] = bv[r1]
        wv[D_MODEL, 64] = 1.0
        wv[D_MODEL, 129] = 1.0

        in_maps.append({
            "xq": xq, "xk": xk, "xv": xv,
            "wq": np.ascontiguousarray(Wq[rows2h].T),
            "wk": np.ascontiguousarray(Wk[rows2h].T),
            "wv": wv,
            "wm0": np.ascontiguousarray(Wm[:, r0].T),
            "wm1": np.ascontiguousarray(Wm[:, r1].T),
            "bq": bq[rows2h][:, None].copy(),
            "bk": bk[rows2h][:, None].copy(),
        })
    meta = (NQ, NK, tuple(Pq.tolist()), tuple(Pk.tolist()))
    return in_maps, meta, qpos, bm


def kernel(query, key, value, query_labels, value_labels,
           Wq, bq, Wk, bk, Wv, bv, Wm, bm):
    from concourse.bass_utils import run_bass_kernel_spmd

    in_maps, meta, qpos, bm_np = _prepare(
        query, key, value, query_labels, value_labels,
        Wq, bq, Wk, bk, Wv, bv, Wm, bm)
    NQ, NK, Pq, Pk = meta
    if meta not in _PROG_CACHE:
        _PROG_CACHE[meta] = _build_program(NQ, NK, np.array(Pq), np.array(Pk))
    nc = _PROG_CACHE[meta]

    res = run_bass_kernel_spmd(nc, in_maps, list(range(8)))

    out = np.empty((B, D_MODEL, N_TOK), np.float32)
    for b in range(B):
        pad = res.results[2 * b]["out"] + res.results[2 * b + 1]["out"]
        out[b] = pad[:, qpos[b]] + bm_np[:, None]
    return out
